# revision 26
# baseline (speedup 1.0000x reference)
"""Trainium2 Bass kernel for nn_CNN3_P (dense_cnn), 8-core data parallel.

Network (per sample):
  x [128,64] -> pairwise conv -> relu -> [256,127]
  -> conv1d k3 (x3, relu) -> [256,121] -> FC 30976->512 relu -> FC 512->1

Strategy: batch 2048 split 256/core. Channels on partitions (2 chunks of
128); conv layers run on a flat compacted layout (per-sample stride
shrinks 127->125->123->121 per layer) so K=3 shifts are plain column
offsets and no garbage columns are streamed. All matmuls fp16 (1 cyc/row
on the PE; fp8 DoubleRow is 2x faster but measured 7-10% rel err on this
net -- far over the 2e-2 budget), PSUM accumulates fp32. Dummy matmuls on
a memset tile at t=0 warm the HAM clock-gate during the initial DMA
window; Wf1 streams through a hoisted 3-buf pool across 3 DMA queues
(sync/gpsimd/scalar, ~330 GB/s) with 3 groups prefetched during the conv
phase so FC1 never stalls; FC1 sample chunks are interleaved (sample =
2p+sc) so the final y store is one contiguous 8B run per partition; FC2
is one DVE scalar_tensor_tensor (relu+mult+row-reduce) per chunk.
"""
import os
import sys

for _p in ('/opt/trn_rl_repo', '/root/.axon_site/_ro/trn_rl_repo'):
    if os.path.isdir(_p) and _p not in sys.path:
        sys.path.insert(0, _p)

import numpy as np

import concourse.bacc as bacc
import concourse.mybir as mybir
import concourse.tile as tile
from concourse.bass_utils import run_bass_kernel_spmd

F32 = mybir.dt.float32
F16 = mybir.dt.float16

P = 128
CL = 128          # context length
IL = 64           # inst length
PC = 256          # channels (all layers)
NCHUNK = 2        # channel chunks of 128
LF = 121          # conv3 valid positions
F1 = 512
N_CORES = 8
B = 2048
BCORE = B // N_CORES      # 256
T = 8                     # samples per conv sub-tile
NT = BCORE // T           # 32
TILE_N = 512              # pairwise psum tile width (4 samples * 128)
SPT = 4                   # samples per psum tile
SC = BCORE // P           # 2 sample chunks of 128 for FC
S0, S1, S2, S3 = 127, 125, 123, 121   # compacted per-sample strides
N_WARMUP = 28             # HAM warmup matmuls at kernel start
GL = 11                   # l-slices per Wf1 DMA (121 = 11*11)

# bisect toggles (default all on)
_WARMUP = os.environ.get("K_WARMUP", "1") == "1"
_PREFETCH = os.environ.get("K_PREFETCH", "1") == "1"
_TTR = os.environ.get("K_TTR", "0") == "1"   # tensor_tensor_reduce crashes on HW
_YDMA1 = os.environ.get("K_YDMA1", "1") == "1"
_STTACC = os.environ.get("K_STTACC", "1") == "1"


def build_nc():
    nc = bacc.Bacc("TRN2", target_bir_lowering=False, debug=False)

    xt_d = nc.dram_tensor("xth", [NT, IL, T * CL], F16, kind="ExternalInput")
    xb_d = nc.dram_tensor("xbh", [NT, IL, T * CL], F16, kind="ExternalInput")
    wpc_d = nc.dram_tensor("wpc", [P, PC], F16, kind="ExternalInput")
    bp_d = nc.dram_tensor("bpc", [NCHUNK, P], F32, kind="ExternalInput")
    wc_d = [nc.dram_tensor(f"w{i}t", [NCHUNK, P, 3 * NCHUNK * P], F16,
                           kind="ExternalInput") for i in (1, 2, 3)]
    bc_d = [nc.dram_tensor(f"b{i}c", [NCHUNK, P], F32, kind="ExternalInput")
            for i in (1, 2, 3)]
    wf1_d = nc.dram_tensor("wf1t", [NCHUNK, LF, P, F1], F16, kind="ExternalInput")
    bf1_d = nc.dram_tensor("bf1r", [1, F1], F16, kind="ExternalInput")
    ones_d = nc.dram_tensor("onesr", [1, P], F16, kind="ExternalInput")
    wf2_d = nc.dram_tensor("wf2b", [P, F1], F16, kind="ExternalInput")
    bf2_d = nc.dram_tensor("bf2c", [P, 1], F32, kind="ExternalInput")
    y_d = nc.dram_tensor("y", [BCORE, 1], F32, kind="ExternalOutput")

    RELU = mybir.ActivationFunctionType.Relu

    with tile.TileContext(nc) as tc:
        with tc.tile_pool(name="const", bufs=1) as cpool, \
             tc.tile_pool(name="h3c", bufs=1) as h3pool, \
             tc.tile_pool(name="wf1", bufs=3) as wfpool, \
             tc.tile_pool(name="h4", bufs=1) as h4pool:
            # --- constants / weights, resident all kernel ---
            wz = cpool.tile([P, P], F16)
            nc.gpsimd.memset(wz[:], 0.0)
            wpc = cpool.tile([P, PC], F16)
            nc.sync.dma_start(wpc[:], wpc_d.ap())
            bp = cpool.tile([P, NCHUNK], F32)
            nc.sync.dma_start(bp[:], bp_d.ap().rearrange("c p -> p c"))
            # conv weights: per layer, per ci-chunk: [ci, (k, coc, co)]
            wconv = []
            for i in range(3):
                tiles = []
                for cic in range(NCHUNK):
                    w = cpool.tile([P, 3 * NCHUNK * P], F16, tag=f"w{i}_{cic}")
                    nc.sync.dma_start(w[:], wc_d[i].ap()[cic])
                    tiles.append(w)
                wconv.append(tiles)
            bconv = []
            for i in range(3):
                bt = cpool.tile([P, NCHUNK], F32, tag=f"bc{i}")
                nc.sync.dma_start(bt[:], bc_d[i].ap().rearrange("c p -> p c"))
                bconv.append(bt)
            bf1 = cpool.tile([1, F1], F16)
            nc.sync.dma_start(bf1[:], bf1_d.ap())
            ones = cpool.tile([1, P], F16)
            nc.sync.dma_start(ones[:], ones_d.ap())
            wf2b = cpool.tile([P, F1], F16)
            nc.sync.dma_start(wf2b[:], wf2_d.ap())
            bf2t = cpool.tile([P, 1], F32)
            nc.sync.dma_start(bf2t[:], bf2_d.ap())

            # persistent conv3 output, fp16, (s, l)-major: col = s*S3 + l
            h3c = [h3pool.tile([P, BCORE * S3], F16, tag=f"h3c{cc}", name=f"h3c{cc}")
                   for cc in range(NCHUNK)]
            h3v = [h.rearrange("p (s l) -> p s l", l=S3) for h in h3c]
            # interleaved sample view for FC1: chunk sc holds samples 2p+sc,
            # so the final y DMA is one contiguous 8B run per partition
            h3i = [h.rearrange("p (s2 two l) -> p two s2 l", two=2, l=S3)
                   for h in h3c]

            # Wf1 stream: prefetch the first two groups at kernel start so
            # FC1 never waits on HBM at the conv->FC transition.
            fc_pairs = [(cc, lg) for cc in range(NCHUNK) for lg in range(LF // GL)]

            # split each group across 4 queues (one DMA engine each) so the
            # FC1 stream sustains ~300 GB/s; per-ll-block chunks let matmuls
            # start as soon as their slice lands
            LLB = [(0, 4), (4, 8), (8, GL)]

            def wf1_fetch(idx):
                cc, lg = fc_pairs[idx]
                rw = wfpool.tile([P, GL * F1], F16, tag="wf1")
                for (a, b), q in zip(LLB, (nc.sync, nc.gpsimd, nc.scalar)):
                    q.dma_start(
                        rw[:, a * F1:b * F1].rearrange("p (l f) -> p l f",
                                                       l=b - a),
                        wf1_d.ap()[cc, lg * GL + a:lg * GL + b].rearrange(
                            "l c f -> c l f"))
                return rw

            # ---------------- conv phase ----------------
            with tc.tile_pool(name="xt", bufs=2) as xtpool, \
                 tc.tile_pool(name="h", bufs=2) as hpool, \
                 tc.tile_pool(name="ps", bufs=8, space="PSUM") as pspool:
                # HAM warmup: the PE clock-gate opens only after ~3.4us of
                # sustained matmul activity; fill the initial DMA-wait window
                # with dummy matmuls so real tiles run at full clock.
                for _ in range(N_WARMUP if _WARMUP else 0):
                    wt = pspool.tile([P, TILE_N], F32, tag="ps", name="warm")
                    nc.tensor.matmul(wt[:, 0:P], wz[:], wz[:],
                                     start=True, stop=True)

                NTS = [1, 0]   # nt=1 first: its consumers don't cross the
                # nt boundary, so they unblock earliest

                def pairwise(t):
                    xt = xtpool.tile([P, T * CL], F16, tag="xt", name="xt")
                    nc.gpsimd.dma_start(xt[0:IL, :], xt_d.ap()[t])
                    # t=0: second half on the scalar queue so both transfer
                    # in parallel and the first pairwise matmul starts sooner
                    q2 = nc.scalar if t == 0 else nc.gpsimd
                    q2.dma_start(xt[IL:P, :], xb_d.ap()[t])
                    h0 = [hpool.tile([P, T * S0], F16, tag=f"h0_{cc}", name=f"h0_{cc}")
                          for cc in range(NCHUNK)]
                    for nt in NTS:
                        for cc in range(NCHUNK):
                            ps = pspool.tile([P, TILE_N], F32, tag="ps", name="pwps")
                            sl_ = slice(nt * TILE_N, (nt + 1) * TILE_N)
                            nc.tensor.matmul(ps[:], wpc[:, cc * P:(cc + 1) * P],
                                             xt[:, sl_], start=True, stop=True)
                            # compact: drop col 0 of each sample (i=0 garbage).
                            # t=0: evacuate on the (idle) DVE so conv1's first
                            # tile isn't serialized behind the ACT queue
                            if t == 0:
                                nc.vector.tensor_scalar(
                                    h0[cc][:, nt * SPT * S0:(nt + 1) * SPT * S0]
                                    .rearrange("p (s i) -> p s i", i=S0),
                                    ps[:].rearrange("p (s i) -> p s i",
                                                    i=CL)[:, :, 1:],
                                    bp[:, cc:cc + 1], 0.0,
                                    mybir.AluOpType.add, mybir.AluOpType.max)
                            else:
                                nc.scalar.activation(
                                    h0[cc][:, nt * SPT * S0:(nt + 1) * SPT * S0]
                                    .rearrange("p (s i) -> p s i", i=CL - 1),
                                    ps[:].rearrange("p (s i) -> p s i",
                                                    i=CL)[:, :, 1:],
                                    RELU, bias=bp[:, cc:cc + 1])
                    return h0

                def conv_layer(hin, w_tiles, s_in, evac):
                    # group-outer: each psum group completes early so its
                    # evacuation overlaps the remaining groups' matmuls
                    n_in = SPT * s_in
                    flat = T * s_in
                    for nt in NTS:
                        for co in range(NCHUNK):
                            ps = pspool.tile([P, TILE_N], F32,
                                             tag="ps", name=f"cps{co}_{nt}")
                            step = 0
                            for k in range(3):
                                for ci in range(NCHUNK):
                                    lhsT = w_tiles[ci][:, (k * NCHUNK + co) * P:
                                                       (k * NCHUNK + co + 1) * P]
                                    nk = min(n_in, flat - nt * n_in - k)
                                    nc.tensor.matmul(
                                        ps[:, 0:nk], lhsT,
                                        hin[ci][:, nt * n_in + k:
                                                nt * n_in + k + nk],
                                        start=(step == 0), stop=(step == 5))
                                    step += 1
                            evac(co, nt, ps)

                h0_next = pairwise(0)
                rw_pre = []
                for t in range(NT):
                    h0 = h0_next
                    h1 = [hpool.tile([P, T * S1], F16, tag=f"h1_{cc}", name=f"h1_{cc}")
                          for cc in range(NCHUNK)]

                    def evac1(co, nt, ps):
                        nc.vector.tensor_scalar(
                            h1[co][:, nt * SPT * S1:(nt + 1) * SPT * S1]
                            .rearrange("p (s i) -> p s i", i=S1),
                            ps[:, 0:SPT * S0]
                            .rearrange("p (s i) -> p s i", i=S0)[:, :, 0:S1],
                            bconv[0][:, co:co + 1], 0.0,
                            mybir.AluOpType.add, mybir.AluOpType.max)
                    conv_layer(h0, wconv[0], S0, evac1)
                    if t == 0 and _PREFETCH:
                        rw_pre = [wf1_fetch(i) for i in range(3)]

                    # emit next tile's pairwise here so its evacuations age
                    # a full tile before conv1(t+1) consumes them
                    if t + 1 < NT:
                        h0_next = pairwise(t + 1)

                    h2 = [hpool.tile([P, T * S2], F16, tag=f"h2_{cc}", name=f"h2_{cc}")
                          for cc in range(NCHUNK)]

                    def evac2(co, nt, ps):
                        nc.vector.tensor_scalar(
                            h2[co][:, nt * SPT * S2:(nt + 1) * SPT * S2]
                            .rearrange("p (s i) -> p s i", i=S2),
                            ps[:, 0:SPT * S1]
                            .rearrange("p (s i) -> p s i", i=S1)[:, :, 0:S2],
                            bconv[1][:, co:co + 1], 0.0,
                            mybir.AluOpType.add, mybir.AluOpType.max)
                    conv_layer(h1, wconv[1], S1, evac2)

                    def evac3(co, nt, ps, t=t):
                        s0 = t * T + nt * SPT
                        nc.scalar.activation(
                            h3c[co][:, s0 * S3:s0 * S3 + SPT * S3]
                            .rearrange("p (s l) -> p s l", l=S3),
                            ps[:, 0:SPT * S2]
                            .rearrange("p (s i) -> p s i", i=S2)[:, :, 0:S3],
                            RELU, bias=bconv[2][:, co:co + 1])
                    conv_layer(h2, wconv[2], S2, evac3)

            # ---------------- FC phase ----------------
            with tc.tile_pool(name="fps", bufs=2, space="PSUM") as fpspool:
                ps_fc1 = [fpspool.tile([P, F1], F32, tag=f"fc1ps{sc}", bufs=1,
                                       name=f"fc1ps{sc}") for sc in range(SC)]
                for sc in range(SC):
                    nc.tensor.matmul(ps_fc1[sc][:], ones[:], bf1[:],
                                     start=True, stop=False)
                for idx in range(len(fc_pairs)):
                    if idx < len(rw_pre):
                        rw = rw_pre[idx]
                    else:
                        rw = wf1_fetch(idx)
                    cc, lg = fc_pairs[idx]
                    for ll in range(GL):
                        l = lg * GL + ll
                        last = (cc == NCHUNK - 1) and (l == LF - 1)
                        for sc in range(SC):
                            nc.tensor.matmul(
                                ps_fc1[sc][:],
                                h3i[cc][:, sc, :, l],
                                rw[:, ll * F1:(ll + 1) * F1],
                                start=False, stop=last)
                # FC2: y = sum_f h4*wf2 + bf2 via one DVE reduce per chunk
                yacc = h4pool.tile([P, SC], F32, tag="yacc", name="yacc")
                yv_d = y_d.ap().rearrange("(p s) one -> p (s one)", s=SC)
                for sc in range(SC):
                    scratch = h4pool.tile([P, F1], F32, tag=f"fc2scr{sc}",
                                          name=f"fc2scr{sc}")
                    if _STTACC:
                        # relu + mult + row-sum in one DVE op from PSUM
                        nc.vector.scalar_tensor_tensor(
                            scratch[:], ps_fc1[sc][:], 0.0, wf2b[:],
                            mybir.AluOpType.max, mybir.AluOpType.mult,
                            accum_out=yacc[:, sc:sc + 1])
                    else:
                        h4 = h4pool.tile([P, F1], F16, tag=f"h4_{sc}",
                                         name=f"h4_{sc}")
                        nc.scalar.activation(h4[:], ps_fc1[sc][:], RELU)
                        nc.vector.tensor_tensor(
                            scratch[:], h4[:], wf2b[:], mybir.AluOpType.mult)
                        nc.vector.tensor_reduce(
                            yacc[:, sc:sc + 1], scratch[:],
                            mybir.AxisListType.X, mybir.AluOpType.add)
                    nc.vector.tensor_scalar_add(
                        yacc[:, sc:sc + 1], yacc[:, sc:sc + 1],
                        bf2t[:, 0:1])
                if _YDMA1:
                    nc.sync.dma_start(yv_d, yacc[:])
                else:
                    for sc in range(SC):
                        nc.sync.dma_start(yv_d[:, sc:sc + 1], yacc[:, sc:sc + 1])

    nc.compile()
    return nc


_NC_CACHE = None


def _get_nc():
    global _NC_CACHE
    if _NC_CACHE is None:
        _NC_CACHE = build_nc()
    return _NC_CACHE


def prep_inputs(x, Wp, bp, W1, b1, W2, b2, W3, b3, Wf1, bf1, Wf2, bf2):
    """Host-side shard + weight re-layout. Returns per-core input maps."""
    f32, f16 = np.float32, np.float16
    wp = np.asarray(Wp, f32)
    wpc = np.ascontiguousarray(
        np.concatenate([wp[:, :, 1].T, wp[:, :, 0].T], axis=0)).astype(f16)
    bpc = np.ascontiguousarray(np.asarray(bp, f32).reshape(NCHUNK, P))

    def conv_t(W):
        # W [co, ci, k] -> [cic, ci, (k, coc, co)] so the DMA is contiguous
        a = np.asarray(W, f32).reshape(NCHUNK, P, NCHUNK, P, 3)
        return np.ascontiguousarray(
            a.transpose(2, 3, 4, 0, 1)).astype(f16).reshape(
                NCHUNK, P, 3 * NCHUNK * P)

    w1t, w2t, w3t = conv_t(W1), conv_t(W2), conv_t(W3)
    b1c = np.ascontiguousarray(np.asarray(b1, f32).reshape(NCHUNK, P))
    b2c = np.ascontiguousarray(np.asarray(b2, f32).reshape(NCHUNK, P))
    b3c = np.ascontiguousarray(np.asarray(b3, f32).reshape(NCHUNK, P))
    # Wf1 [512, 30976] -> [cc, l, c, f] fp16 (contiguous 128KB per (cc, l))
    wf1t = np.ascontiguousarray(
        np.asarray(Wf1, f32).reshape(F1, NCHUNK, P, LF)
        .transpose(1, 3, 2, 0)).astype(f16)
    bf1r = np.ascontiguousarray(np.asarray(bf1, f32).reshape(1, F1)).astype(f16)
    onesr = np.ones((1, P), f16)
    wf2b = np.ascontiguousarray(
        np.broadcast_to(np.asarray(Wf2, f32).reshape(1, F1), (P, F1))).astype(f16)
    bf2c = np.ascontiguousarray(
        np.broadcast_to(np.asarray(bf2, f32).reshape(1, 1), (P, 1)))

    shared = dict(wpc=wpc, bpc=bpc, w1t=w1t, w2t=w2t, w3t=w3t,
                  b1c=b1c, b2c=b2c, b3c=b3c, wf1t=wf1t, bf1r=bf1r,
                  wf2b=wf2b, bf2c=bf2c, onesr=onesr)
    xr = np.asarray(x, f32).reshape(N_CORES, NT, T, CL, IL).astype(f16)
    # [nc, t, j, s, i]: per (t, j) a contiguous 8*128 run -> 64x2KB DMAs
    xtt = np.ascontiguousarray(xr.transpose(0, 1, 4, 2, 3))
    xbt = np.ascontiguousarray(
        np.broadcast_to(xtt[:, :, :, :, 0:1], xtt.shape))           # x0 repl over i
    xth = xtt.reshape(N_CORES, NT, IL, T * CL)
    xbh = xbt.reshape(N_CORES, NT, IL, T * CL)
    return [dict(xth=xth[i], xbh=xbh[i], **shared) for i in range(N_CORES)]


def kernel(x, Wp, bp, W1, b1, W2, b2, W3, b3, Wf1, bf1, Wf2, bf2,
           trace=False, **run_kwargs):
    nc = _get_nc()
    in_maps = prep_inputs(x, Wp, bp, W1, b1, W2, b2, W3, b3, Wf1, bf1, Wf2, bf2)
    res = run_bass_kernel_spmd(nc, in_maps, core_ids=list(range(N_CORES)),
                               trace=trace, **run_kwargs)
    out = np.concatenate([res.results[i]["y"] for i in range(N_CORES)], axis=0)
    kernel.last_results = res
    return out.astype(np.float32)


kernel.last_results = None


# revision 27
# speedup vs baseline: 1.1906x; 1.1906x over previous
"""Trainium2 Bass kernel for nn_CNN3_P (dense_cnn), 8-core data parallel.

Network (per sample):
  x [128,64] -> pairwise conv -> relu -> [256,127]
  -> conv1d k3 (x3, relu) -> [256,121] -> FC 30976->512 relu -> FC 512->1

Strategy: batch 2048 split 256/core. Channels on partitions (2 chunks of
128); conv layers run on a flat compacted layout (per-sample stride
shrinks 127->125->123->121 per layer) so K=3 shifts are plain column
offsets and no garbage columns are streamed. All matmuls fp16 (1 cyc/row
on the PE; fp8 DoubleRow is 2x faster but measured 7-10% rel err on this
net -- far over the 2e-2 budget), PSUM accumulates fp32. Dummy matmuls on
a memset tile at t=0 warm the HAM clock-gate during the initial DMA
window; Wf1 streams through a hoisted 3-buf pool across 3 DMA queues
(sync/gpsimd/scalar, ~330 GB/s) with 3 groups prefetched during the conv
phase so FC1 never stalls; FC1 sample chunks are interleaved (sample =
2p+sc) so the final y store is one contiguous 8B run per partition; FC2
is one DVE scalar_tensor_tensor (relu+mult+row-reduce) per chunk.
"""
import os
import sys

for _p in ('/opt/trn_rl_repo', '/root/.axon_site/_ro/trn_rl_repo'):
    if os.path.isdir(_p) and _p not in sys.path:
        sys.path.insert(0, _p)

import numpy as np

import concourse.bacc as bacc
import concourse.mybir as mybir
import concourse.tile as tile
from concourse.bass_utils import run_bass_kernel_spmd

F32 = mybir.dt.float32
F16 = mybir.dt.float16

P = 128
CL = 128          # context length
IL = 64           # inst length
PC = 256          # channels (all layers)
NCHUNK = 2        # channel chunks of 128
LF = 121          # conv3 valid positions
F1 = 512
N_CORES = 8
B = 2048
BCORE = B // N_CORES      # 256
T = 8                     # samples per conv sub-tile
NT = BCORE // T           # 32
TILE_N = 512              # pairwise psum tile width (4 samples * 128)
SPT = 4                   # samples per psum tile
SC = BCORE // P           # 2 sample chunks of 128 for FC
S0, S1, S2, S3 = 127, 125, 123, 121   # compacted per-sample strides
N_WARMUP = 28             # HAM warmup matmuls at kernel start
GL = 11                   # l-slices per Wf1 DMA (121 = 11*11)

# bisect toggles (default all on)
_WARMUP = os.environ.get("K_WARMUP", "1") == "1"
_PREFETCH = os.environ.get("K_PREFETCH", "1") == "1"
_TTR = os.environ.get("K_TTR", "0") == "1"   # tensor_tensor_reduce crashes on HW
_YDMA1 = os.environ.get("K_YDMA1", "1") == "1"
_STTACC = os.environ.get("K_STTACC", "1") == "1"


def build_nc():
    nc = bacc.Bacc("TRN2", target_bir_lowering=False, debug=False)

    xt_d = nc.dram_tensor("xth", [NT, IL, T * CL], F16, kind="ExternalInput")
    xb_d = nc.dram_tensor("xbh", [NT, IL, T * CL], F16, kind="ExternalInput")
    wpc_d = nc.dram_tensor("wpc", [P, PC], F16, kind="ExternalInput")
    bp_d = nc.dram_tensor("bpc", [NCHUNK, P], F32, kind="ExternalInput")
    wc_d = [nc.dram_tensor(f"w{i}t", [NCHUNK, P, 3 * NCHUNK * P], F16,
                           kind="ExternalInput") for i in (1, 2, 3)]
    bc_d = [nc.dram_tensor(f"b{i}c", [NCHUNK, P], F32, kind="ExternalInput")
            for i in (1, 2, 3)]
    wf1_d = nc.dram_tensor("wf1t", [NCHUNK, LF, P, F1], F16, kind="ExternalInput")
    bf1_d = nc.dram_tensor("bf1r", [1, F1], F16, kind="ExternalInput")
    ones_d = nc.dram_tensor("onesr", [1, P], F16, kind="ExternalInput")
    wf2_d = nc.dram_tensor("wf2b", [P, F1], F16, kind="ExternalInput")
    bf2_d = nc.dram_tensor("bf2c", [P, 1], F32, kind="ExternalInput")
    y_d = nc.dram_tensor("y", [BCORE, 1], F32, kind="ExternalOutput")

    RELU = mybir.ActivationFunctionType.Relu

    with tile.TileContext(nc) as tc:
        with tc.tile_pool(name="const", bufs=1) as cpool, \
             tc.tile_pool(name="h3c", bufs=1) as h3pool, \
             tc.tile_pool(name="wf1", bufs=3) as wfpool, \
             tc.tile_pool(name="h4", bufs=1) as h4pool:
            # --- constants / weights, resident all kernel ---
            wz = cpool.tile([P, P], F16)
            nc.gpsimd.memset(wz[:], 0.0)
            wpc = cpool.tile([P, PC], F16)
            nc.sync.dma_start(wpc[:], wpc_d.ap())
            bp = cpool.tile([P, NCHUNK], F32)
            nc.sync.dma_start(bp[:], bp_d.ap().rearrange("c p -> p c"))
            # conv weights: per layer, per ci-chunk: [ci, (k, coc, co)]
            wconv = []
            for i in range(3):
                tiles = []
                for cic in range(NCHUNK):
                    w = cpool.tile([P, 3 * NCHUNK * P], F16, tag=f"w{i}_{cic}")
                    nc.sync.dma_start(w[:], wc_d[i].ap()[cic])
                    tiles.append(w)
                wconv.append(tiles)
            bconv = []
            for i in range(3):
                bt = cpool.tile([P, NCHUNK], F32, tag=f"bc{i}")
                nc.sync.dma_start(bt[:], bc_d[i].ap().rearrange("c p -> p c"))
                bconv.append(bt)
            bf1 = cpool.tile([1, F1], F16)
            nc.sync.dma_start(bf1[:], bf1_d.ap())
            ones = cpool.tile([1, P], F16)
            nc.sync.dma_start(ones[:], ones_d.ap())
            wf2b = cpool.tile([P, F1], F16)
            nc.sync.dma_start(wf2b[:], wf2_d.ap())
            bf2t = cpool.tile([P, 1], F32)
            nc.sync.dma_start(bf2t[:], bf2_d.ap())

            # persistent conv3 output, fp16, (s, l)-major: col = s*S3 + l
            h3c = [h3pool.tile([P, BCORE * S3], F16, tag=f"h3c{cc}", name=f"h3c{cc}")
                   for cc in range(NCHUNK)]
            h3v = [h.rearrange("p (s l) -> p s l", l=S3) for h in h3c]
            # interleaved sample view for FC1: chunk sc holds samples 2p+sc,
            # so the final y DMA is one contiguous 8B run per partition
            h3i = [h.rearrange("p (s2 two l) -> p two s2 l", two=2, l=S3)
                   for h in h3c]

            # Wf1 stream: prefetch the first two groups at kernel start so
            # FC1 never waits on HBM at the conv->FC transition.
            fc_pairs = [(cc, lg) for cc in range(NCHUNK) for lg in range(LF // GL)]

            # split each group across 4 queues (one DMA engine each) so the
            # FC1 stream sustains ~300 GB/s; per-ll-block chunks let matmuls
            # start as soon as their slice lands
            LLB = [(0, 4), (4, 8), (8, GL)]

            def wf1_fetch(idx):
                cc, lg = fc_pairs[idx]
                rw = wfpool.tile([P, GL * F1], F16, tag="wf1")
                for (a, b), q in zip(LLB, (nc.sync, nc.gpsimd, nc.scalar)):
                    q.dma_start(
                        rw[:, a * F1:b * F1].rearrange("p (l f) -> p l f",
                                                       l=b - a),
                        wf1_d.ap()[cc, lg * GL + a:lg * GL + b].rearrange(
                            "l c f -> c l f"))
                return rw

            # ---------------- conv phase ----------------
            with tc.tile_pool(name="xt", bufs=2) as xtpool, \
                 tc.tile_pool(name="h", bufs=2) as hpool, \
                 tc.tile_pool(name="ps", bufs=8, space="PSUM") as pspool:
                # HAM warmup: the PE clock-gate opens only after ~3.4us of
                # sustained matmul activity; fill the initial DMA-wait window
                # with dummy matmuls so real tiles run at full clock.
                for _ in range(N_WARMUP if _WARMUP else 0):
                    wt = pspool.tile([P, TILE_N], F32, tag="ps", name="warm")
                    nc.tensor.matmul(wt[:, 0:P], wz[:], wz[:],
                                     start=True, stop=True)

                NTS = [1, 0]   # nt=1 first: its consumers don't cross the
                # nt boundary, so they unblock earliest

                def pairwise(t):
                    xt = xtpool.tile([P, T * CL], F16, tag="xt", name="xt")
                    nc.gpsimd.dma_start(xt[0:IL, :], xt_d.ap()[t])
                    # t=0: second half on the scalar queue so both transfer
                    # in parallel and the first pairwise matmul starts sooner
                    q2 = nc.scalar if t == 0 else nc.gpsimd
                    q2.dma_start(xt[IL:P, :], xb_d.ap()[t])
                    h0 = [hpool.tile([P, T * S0], F16, tag=f"h0_{cc}", name=f"h0_{cc}")
                          for cc in range(NCHUNK)]
                    for nt in NTS:
                        for cc in range(NCHUNK):
                            ps = pspool.tile([P, TILE_N], F32, tag="ps", name="pwps")
                            sl_ = slice(nt * TILE_N, (nt + 1) * TILE_N)
                            nc.tensor.matmul(ps[:], wpc[:, cc * P:(cc + 1) * P],
                                             xt[:, sl_], start=True, stop=True)
                            # compact: drop col 0 of each sample (i=0 garbage).
                            # t=0: nt=1 evacuates on the (idle) DVE so conv1's
                            # first tile isn't serialized behind the ACT queue
                            if t == 0 and nt == 1:
                                nc.vector.tensor_scalar(
                                    h0[cc][:, nt * SPT * S0:(nt + 1) * SPT * S0]
                                    .rearrange("p (s i) -> p s i", i=S0),
                                    ps[:].rearrange("p (s i) -> p s i",
                                                    i=CL)[:, :, 1:],
                                    bp[:, cc:cc + 1], 0.0,
                                    mybir.AluOpType.add, mybir.AluOpType.max)
                            else:
                                nc.scalar.activation(
                                    h0[cc][:, nt * SPT * S0:(nt + 1) * SPT * S0]
                                    .rearrange("p (s i) -> p s i", i=CL - 1),
                                    ps[:].rearrange("p (s i) -> p s i",
                                                    i=CL)[:, :, 1:],
                                    RELU, bias=bp[:, cc:cc + 1])
                    return h0

                def conv_layer(hin, w_tiles, s_in, evac):
                    # group-outer: each psum group completes early so its
                    # evacuation overlaps the remaining groups' matmuls
                    n_in = SPT * s_in
                    flat = T * s_in
                    for nt in NTS:
                        for co in range(NCHUNK):
                            ps = pspool.tile([P, TILE_N], F32,
                                             tag="ps", name=f"cps{co}_{nt}")
                            step = 0
                            for k in range(3):
                                for ci in range(NCHUNK):
                                    lhsT = w_tiles[ci][:, (k * NCHUNK + co) * P:
                                                       (k * NCHUNK + co + 1) * P]
                                    nk = min(n_in, flat - nt * n_in - k)
                                    nc.tensor.matmul(
                                        ps[:, 0:nk], lhsT,
                                        hin[ci][:, nt * n_in + k:
                                                nt * n_in + k + nk],
                                        start=(step == 0), stop=(step == 5))
                                    step += 1
                            evac(co, nt, ps)

                h0_next = pairwise(0)
                rw_pre = []
                for t in range(NT):
                    h0 = h0_next
                    h1 = [hpool.tile([P, T * S1], F16, tag=f"h1_{cc}", name=f"h1_{cc}")
                          for cc in range(NCHUNK)]

                    def evac1(co, nt, ps):
                        nc.vector.tensor_scalar(
                            h1[co][:, nt * SPT * S1:(nt + 1) * SPT * S1]
                            .rearrange("p (s i) -> p s i", i=S1),
                            ps[:, 0:SPT * S0]
                            .rearrange("p (s i) -> p s i", i=S0)[:, :, 0:S1],
                            bconv[0][:, co:co + 1], 0.0,
                            mybir.AluOpType.add, mybir.AluOpType.max)
                    conv_layer(h0, wconv[0], S0, evac1)
                    if t == 0 and _PREFETCH:
                        rw_pre = [wf1_fetch(i) for i in range(3)]

                    # emit next tile's pairwise here so its evacuations age
                    # a full tile before conv1(t+1) consumes them
                    if t + 1 < NT:
                        h0_next = pairwise(t + 1)

                    h2 = [hpool.tile([P, T * S2], F16, tag=f"h2_{cc}", name=f"h2_{cc}")
                          for cc in range(NCHUNK)]

                    def evac2(co, nt, ps):
                        nc.vector.tensor_scalar(
                            h2[co][:, nt * SPT * S2:(nt + 1) * SPT * S2]
                            .rearrange("p (s i) -> p s i", i=S2),
                            ps[:, 0:SPT * S1]
                            .rearrange("p (s i) -> p s i", i=S1)[:, :, 0:S2],
                            bconv[1][:, co:co + 1], 0.0,
                            mybir.AluOpType.add, mybir.AluOpType.max)
                    conv_layer(h1, wconv[1], S1, evac2)

                    def evac3(co, nt, ps, t=t):
                        s0 = t * T + nt * SPT
                        nc.scalar.activation(
                            h3c[co][:, s0 * S3:s0 * S3 + SPT * S3]
                            .rearrange("p (s l) -> p s l", l=S3),
                            ps[:, 0:SPT * S2]
                            .rearrange("p (s i) -> p s i", i=S2)[:, :, 0:S3],
                            RELU, bias=bconv[2][:, co:co + 1])
                    conv_layer(h2, wconv[2], S2, evac3)

            # ---------------- FC phase ----------------
            with tc.tile_pool(name="fps", bufs=2, space="PSUM") as fpspool:
                ps_fc1 = [fpspool.tile([P, F1], F32, tag=f"fc1ps{sc}", bufs=1,
                                       name=f"fc1ps{sc}") for sc in range(SC)]
                for sc in range(SC):
                    nc.tensor.matmul(ps_fc1[sc][:], ones[:], bf1[:],
                                     start=True, stop=False)
                for idx in range(len(fc_pairs)):
                    if idx < len(rw_pre):
                        rw = rw_pre[idx]
                    else:
                        rw = wf1_fetch(idx)
                    cc, lg = fc_pairs[idx]
                    for ll in range(GL):
                        l = lg * GL + ll
                        last = (cc == NCHUNK - 1) and (l == LF - 1)
                        for sc in range(SC):
                            nc.tensor.matmul(
                                ps_fc1[sc][:],
                                h3i[cc][:, sc, :, l],
                                rw[:, ll * F1:(ll + 1) * F1],
                                start=False, stop=last)
                # FC2: y = sum_f h4*wf2 + bf2 via one DVE reduce per chunk
                yacc = h4pool.tile([P, SC], F32, tag="yacc", name="yacc")
                yv_d = y_d.ap().rearrange("(p s) one -> p (s one)", s=SC)
                for sc in range(SC):
                    scratch = h4pool.tile([P, F1], F32, tag=f"fc2scr{sc}",
                                          name=f"fc2scr{sc}")
                    if _STTACC:
                        # relu + mult + row-sum in one DVE op from PSUM
                        nc.vector.scalar_tensor_tensor(
                            scratch[:], ps_fc1[sc][:], 0.0, wf2b[:],
                            mybir.AluOpType.max, mybir.AluOpType.mult,
                            accum_out=yacc[:, sc:sc + 1])
                    else:
                        h4 = h4pool.tile([P, F1], F16, tag=f"h4_{sc}",
                                         name=f"h4_{sc}")
                        nc.scalar.activation(h4[:], ps_fc1[sc][:], RELU)
                        nc.vector.tensor_tensor(
                            scratch[:], h4[:], wf2b[:], mybir.AluOpType.mult)
                        nc.vector.tensor_reduce(
                            yacc[:, sc:sc + 1], scratch[:],
                            mybir.AxisListType.X, mybir.AluOpType.add)
                    nc.vector.tensor_scalar_add(
                        yacc[:, sc:sc + 1], yacc[:, sc:sc + 1],
                        bf2t[:, 0:1])
                if _YDMA1:
                    nc.sync.dma_start(yv_d, yacc[:])
                else:
                    for sc in range(SC):
                        nc.sync.dma_start(yv_d[:, sc:sc + 1], yacc[:, sc:sc + 1])

    nc.compile()
    return nc


_NC_CACHE = None


def _get_nc():
    global _NC_CACHE
    if _NC_CACHE is None:
        _NC_CACHE = build_nc()
    return _NC_CACHE


def prep_inputs(x, Wp, bp, W1, b1, W2, b2, W3, b3, Wf1, bf1, Wf2, bf2):
    """Host-side shard + weight re-layout. Returns per-core input maps."""
    f32, f16 = np.float32, np.float16
    wp = np.asarray(Wp, f32)
    wpc = np.ascontiguousarray(
        np.concatenate([wp[:, :, 1].T, wp[:, :, 0].T], axis=0)).astype(f16)
    bpc = np.ascontiguousarray(np.asarray(bp, f32).reshape(NCHUNK, P))

    def conv_t(W):
        # W [co, ci, k] -> [cic, ci, (k, coc, co)] so the DMA is contiguous
        a = np.asarray(W, f32).reshape(NCHUNK, P, NCHUNK, P, 3)
        return np.ascontiguousarray(
            a.transpose(2, 3, 4, 0, 1)).astype(f16).reshape(
                NCHUNK, P, 3 * NCHUNK * P)

    w1t, w2t, w3t = conv_t(W1), conv_t(W2), conv_t(W3)
    b1c = np.ascontiguousarray(np.asarray(b1, f32).reshape(NCHUNK, P))
    b2c = np.ascontiguousarray(np.asarray(b2, f32).reshape(NCHUNK, P))
    b3c = np.ascontiguousarray(np.asarray(b3, f32).reshape(NCHUNK, P))
    # Wf1 [512, 30976] -> [cc, l, c, f] fp16 (contiguous 128KB per (cc, l))
    wf1t = np.ascontiguousarray(
        np.asarray(Wf1, f32).reshape(F1, NCHUNK, P, LF)
        .transpose(1, 3, 2, 0)).astype(f16)
    bf1r = np.ascontiguousarray(np.asarray(bf1, f32).reshape(1, F1)).astype(f16)
    onesr = np.ones((1, P), f16)
    wf2b = np.ascontiguousarray(
        np.broadcast_to(np.asarray(Wf2, f32).reshape(1, F1), (P, F1))).astype(f16)
    bf2c = np.ascontiguousarray(
        np.broadcast_to(np.asarray(bf2, f32).reshape(1, 1), (P, 1)))

    shared = dict(wpc=wpc, bpc=bpc, w1t=w1t, w2t=w2t, w3t=w3t,
                  b1c=b1c, b2c=b2c, b3c=b3c, wf1t=wf1t, bf1r=bf1r,
                  wf2b=wf2b, bf2c=bf2c, onesr=onesr)
    xr = np.asarray(x, f32).reshape(N_CORES, NT, T, CL, IL).astype(f16)
    # [nc, t, j, s, i]: per (t, j) a contiguous 8*128 run -> 64x2KB DMAs
    xtt = np.ascontiguousarray(xr.transpose(0, 1, 4, 2, 3))
    xbt = np.ascontiguousarray(
        np.broadcast_to(xtt[:, :, :, :, 0:1], xtt.shape))           # x0 repl over i
    xth = xtt.reshape(N_CORES, NT, IL, T * CL)
    xbh = xbt.reshape(N_CORES, NT, IL, T * CL)
    return [dict(xth=xth[i], xbh=xbh[i], **shared) for i in range(N_CORES)]


def kernel(x, Wp, bp, W1, b1, W2, b2, W3, b3, Wf1, bf1, Wf2, bf2,
           trace=False, **run_kwargs):
    nc = _get_nc()
    in_maps = prep_inputs(x, Wp, bp, W1, b1, W2, b2, W3, b3, Wf1, bf1, Wf2, bf2)
    res = run_bass_kernel_spmd(nc, in_maps, core_ids=list(range(N_CORES)),
                               trace=trace, **run_kwargs)
    out = np.concatenate([res.results[i]["y"] for i in range(N_CORES)], axis=0)
    kernel.last_results = res
    return out.astype(np.float32)


kernel.last_results = None


# revision 28
# speedup vs baseline: 1.1977x; 1.0060x over previous
"""Trainium2 Bass kernel for nn_CNN3_P (dense_cnn), 8-core data parallel.

Network (per sample):
  x [128,64] -> pairwise conv -> relu -> [256,127]
  -> conv1d k3 (x3, relu) -> [256,121] -> FC 30976->512 relu -> FC 512->1

Strategy: batch 2048 split 256/core. Channels on partitions (2 chunks of
128); conv layers run on a flat compacted layout (per-sample stride
shrinks 127->125->123->121 per layer) so K=3 shifts are plain column
offsets and no garbage columns are streamed. All matmuls fp16 (1 cyc/row
on the PE; fp8 DoubleRow is 2x faster but measured 7-10% rel err on this
net -- far over the 2e-2 budget), PSUM accumulates fp32. Dummy matmuls on
a memset tile at t=0 warm the HAM clock-gate during the initial DMA
window; Wf1 streams through a hoisted 3-buf pool across 3 DMA queues
(sync/gpsimd/scalar, ~330 GB/s) with 3 groups prefetched during the conv
phase so FC1 never stalls; FC1 sample chunks are interleaved (sample =
2p+sc) so the final y store is one contiguous 8B run per partition; FC2
is one DVE scalar_tensor_tensor (relu+mult+row-reduce) per chunk.
"""
import os
import sys

for _p in ('/opt/trn_rl_repo', '/root/.axon_site/_ro/trn_rl_repo'):
    if os.path.isdir(_p) and _p not in sys.path:
        sys.path.insert(0, _p)

import numpy as np

import concourse.bacc as bacc
import concourse.mybir as mybir
import concourse.tile as tile
from concourse.bass_utils import run_bass_kernel_spmd

F32 = mybir.dt.float32
F16 = mybir.dt.float16

P = 128
CL = 128          # context length
IL = 64           # inst length
PC = 256          # channels (all layers)
NCHUNK = 2        # channel chunks of 128
LF = 121          # conv3 valid positions
F1 = 512
N_CORES = 8
B = 2048
BCORE = B // N_CORES      # 256
T = 8                     # samples per conv sub-tile
NT = BCORE // T           # 32
TILE_N = 512              # pairwise psum tile width (4 samples * 128)
SPT = 4                   # samples per psum tile
SC = BCORE // P           # 2 sample chunks of 128 for FC
S0, S1, S2, S3 = 127, 125, 123, 121   # compacted per-sample strides
N_WARMUP = 28             # HAM warmup matmuls at kernel start
GL = 11                   # l-slices per Wf1 DMA (121 = 11*11)

# bisect toggles (default all on)
_WARMUP = os.environ.get("K_WARMUP", "1") == "1"
_PREFETCH = os.environ.get("K_PREFETCH", "1") == "1"
_TTR = os.environ.get("K_TTR", "0") == "1"   # tensor_tensor_reduce crashes on HW
_YDMA1 = os.environ.get("K_YDMA1", "1") == "1"
_STTACC = os.environ.get("K_STTACC", "1") == "1"


def build_nc():
    nc = bacc.Bacc("TRN2", target_bir_lowering=False, debug=False)

    xt_d = nc.dram_tensor("xth", [NT, IL, T * CL], F16, kind="ExternalInput")
    xb_d = nc.dram_tensor("xbh", [NT, IL, T * CL], F16, kind="ExternalInput")
    wpc_d = nc.dram_tensor("wpc", [P, PC], F16, kind="ExternalInput")
    bp_d = nc.dram_tensor("bpc", [NCHUNK, P], F32, kind="ExternalInput")
    wc_d = [nc.dram_tensor(f"w{i}t", [NCHUNK, P, 3 * NCHUNK * P], F16,
                           kind="ExternalInput") for i in (1, 2, 3)]
    bc_d = [nc.dram_tensor(f"b{i}c", [NCHUNK, P], F32, kind="ExternalInput")
            for i in (1, 2, 3)]
    wf1_d = nc.dram_tensor("wf1t", [NCHUNK, LF, P, F1], F16, kind="ExternalInput")
    bf1_d = nc.dram_tensor("bf1r", [1, F1], F16, kind="ExternalInput")
    ones_d = nc.dram_tensor("onesr", [1, P], F16, kind="ExternalInput")
    wf2_d = nc.dram_tensor("wf2b", [P, F1], F16, kind="ExternalInput")
    bf2_d = nc.dram_tensor("bf2c", [P, 1], F32, kind="ExternalInput")
    y_d = nc.dram_tensor("y", [BCORE, 1], F32, kind="ExternalOutput")

    RELU = mybir.ActivationFunctionType.Relu

    with tile.TileContext(nc) as tc:
        with tc.tile_pool(name="const", bufs=1) as cpool, \
             tc.tile_pool(name="h3c", bufs=1) as h3pool, \
             tc.tile_pool(name="wf1", bufs=3) as wfpool, \
             tc.tile_pool(name="h4", bufs=1) as h4pool:
            # --- constants / weights, resident all kernel ---
            wz = cpool.tile([P, P], F16)
            nc.gpsimd.memset(wz[:], 0.0)
            wpc = cpool.tile([P, PC], F16)
            nc.sync.dma_start(wpc[:], wpc_d.ap())
            bp = cpool.tile([P, NCHUNK], F32)
            nc.sync.dma_start(bp[:], bp_d.ap().rearrange("c p -> p c"))
            # conv weights: per layer, per ci-chunk: [ci, (k, coc, co)]
            wconv = []
            for i in range(3):
                tiles = []
                for cic in range(NCHUNK):
                    w = cpool.tile([P, 3 * NCHUNK * P], F16, tag=f"w{i}_{cic}")
                    nc.sync.dma_start(w[:], wc_d[i].ap()[cic])
                    tiles.append(w)
                wconv.append(tiles)
            bconv = []
            for i in range(3):
                bt = cpool.tile([P, NCHUNK], F32, tag=f"bc{i}")
                nc.sync.dma_start(bt[:], bc_d[i].ap().rearrange("c p -> p c"))
                bconv.append(bt)
            bf1 = cpool.tile([1, F1], F16)
            nc.sync.dma_start(bf1[:], bf1_d.ap())
            ones = cpool.tile([1, P], F16)
            nc.sync.dma_start(ones[:], ones_d.ap())
            wf2b = cpool.tile([P, F1], F16)
            nc.sync.dma_start(wf2b[:], wf2_d.ap())
            bf2t = cpool.tile([P, 1], F32)
            nc.sync.dma_start(bf2t[:], bf2_d.ap())

            # persistent conv3 output, fp16, (s, l)-major: col = s*S3 + l
            h3c = [h3pool.tile([P, BCORE * S3], F16, tag=f"h3c{cc}", name=f"h3c{cc}")
                   for cc in range(NCHUNK)]
            h3v = [h.rearrange("p (s l) -> p s l", l=S3) for h in h3c]
            # interleaved sample view for FC1: chunk sc holds samples 2p+sc,
            # so the final y DMA is one contiguous 8B run per partition
            h3i = [h.rearrange("p (s2 two l) -> p two s2 l", two=2, l=S3)
                   for h in h3c]

            # Wf1 stream: prefetch the first two groups at kernel start so
            # FC1 never waits on HBM at the conv->FC transition.
            fc_pairs = [(cc, lg) for cc in range(NCHUNK) for lg in range(LF // GL)]

            # split each group across 4 queues (one DMA engine each) so the
            # FC1 stream sustains ~300 GB/s; per-ll-block chunks let matmuls
            # start as soon as their slice lands
            LLB = [(0, 4), (4, 8), (8, GL)]

            def wf1_fetch(idx):
                cc, lg = fc_pairs[idx]
                rw = wfpool.tile([P, GL * F1], F16, tag="wf1")
                for (a, b), q in zip(LLB, (nc.sync, nc.gpsimd, nc.scalar)):
                    q.dma_start(
                        rw[:, a * F1:b * F1].rearrange("p (l f) -> p l f",
                                                       l=b - a),
                        wf1_d.ap()[cc, lg * GL + a:lg * GL + b].rearrange(
                            "l c f -> c l f"))
                return rw

            # ---------------- conv phase ----------------
            with tc.tile_pool(name="xt", bufs=2) as xtpool, \
                 tc.tile_pool(name="h", bufs=2) as hpool, \
                 tc.tile_pool(name="ps", bufs=8, space="PSUM") as pspool:
                # HAM warmup: the PE clock-gate opens only after ~3.4us of
                # sustained matmul activity; fill the initial DMA-wait window
                # with dummy matmuls so real tiles run at full clock.
                for _ in range(N_WARMUP if _WARMUP else 0):
                    wt = pspool.tile([P, TILE_N], F32, tag="ps", name="warm")
                    nc.tensor.matmul(wt[:, 0:P], wz[:], wz[:],
                                     start=True, stop=True)

                NTS = [1, 0]   # nt=1 first: its consumers don't cross the
                # nt boundary, so they unblock earliest

                def pairwise(t):
                    xt = xtpool.tile([P, T * CL], F16, tag="xt", name="xt")
                    nc.gpsimd.dma_start(xt[0:IL, :], xt_d.ap()[t])
                    # t=0: second half on the scalar queue so both transfer
                    # in parallel and the first pairwise matmul starts sooner
                    q2 = nc.scalar if t == 0 else nc.gpsimd
                    q2.dma_start(xt[IL:P, :], xb_d.ap()[t])
                    h0 = [hpool.tile([P, T * S0], F16, tag=f"h0_{cc}", name=f"h0_{cc}")
                          for cc in range(NCHUNK)]
                    for nt in NTS:
                        for cc in range(NCHUNK):
                            ps = pspool.tile([P, TILE_N], F32, tag="ps", name="pwps")
                            sl_ = slice(nt * TILE_N, (nt + 1) * TILE_N)
                            nc.tensor.matmul(ps[:], wpc[:, cc * P:(cc + 1) * P],
                                             xt[:, sl_], start=True, stop=True)
                            # compact: drop col 0 of each sample (i=0 garbage).
                            # t=0: nt=1 evacuates on the (idle) DVE so conv1's
                            # first tile isn't serialized behind the ACT queue
                            if t == 0 and nt == 1:
                                nc.vector.tensor_scalar(
                                    h0[cc][:, nt * SPT * S0:(nt + 1) * SPT * S0]
                                    .rearrange("p (s i) -> p s i", i=S0),
                                    ps[:].rearrange("p (s i) -> p s i",
                                                    i=CL)[:, :, 1:],
                                    bp[:, cc:cc + 1], 0.0,
                                    mybir.AluOpType.add, mybir.AluOpType.max)
                            else:
                                nc.scalar.activation(
                                    h0[cc][:, nt * SPT * S0:(nt + 1) * SPT * S0]
                                    .rearrange("p (s i) -> p s i", i=CL - 1),
                                    ps[:].rearrange("p (s i) -> p s i",
                                                    i=CL)[:, :, 1:],
                                    RELU, bias=bp[:, cc:cc + 1])
                    return h0

                def conv_layer(hin, w_tiles, s_in, evac):
                    # group-outer: each psum group completes early so its
                    # evacuation overlaps the remaining groups' matmuls.
                    # moving operand is a [128, 4, s_out] view: only valid
                    # output columns are streamed (no boundary garbage)
                    s_out = s_in - 2
                    n_in = SPT * s_in
                    hv = [h.rearrange("p (s i) -> p s i", i=s_in) for h in hin]
                    for nt in NTS:
                        for co in range(NCHUNK):
                            ps = pspool.tile([P, TILE_N], F32,
                                             tag="ps", name=f"cps{co}_{nt}")
                            step = 0
                            for k in range(3):
                                for ci in range(NCHUNK):
                                    lhsT = w_tiles[ci][:, (k * NCHUNK + co) * P:
                                                       (k * NCHUNK + co + 1) * P]
                                    nc.tensor.matmul(
                                        ps[:, 0:SPT * s_out], lhsT,
                                        hv[ci][:, nt * SPT:(nt + 1) * SPT,
                                               k:k + s_out],
                                        start=(step == 0), stop=(step == 5))
                                    step += 1
                            evac(co, nt, ps)

                h0_next = pairwise(0)
                rw_pre = []
                for t in range(NT):
                    h0 = h0_next
                    h1 = [hpool.tile([P, T * S1], F16, tag=f"h1_{cc}", name=f"h1_{cc}")
                          for cc in range(NCHUNK)]

                    def evac1(co, nt, ps):
                        nc.vector.tensor_scalar(
                            h1[co][:, nt * SPT * S1:(nt + 1) * SPT * S1],
                            ps[:, 0:SPT * S1],
                            bconv[0][:, co:co + 1], 0.0,
                            mybir.AluOpType.add, mybir.AluOpType.max)
                    conv_layer(h0, wconv[0], S0, evac1)
                    if t == 0 and _PREFETCH:
                        rw_pre = [wf1_fetch(i) for i in range(3)]

                    # emit next tile's pairwise here so its evacuations age
                    # a full tile before conv1(t+1) consumes them
                    if t + 1 < NT:
                        h0_next = pairwise(t + 1)

                    h2 = [hpool.tile([P, T * S2], F16, tag=f"h2_{cc}", name=f"h2_{cc}")
                          for cc in range(NCHUNK)]

                    def evac2(co, nt, ps):
                        nc.vector.tensor_scalar(
                            h2[co][:, nt * SPT * S2:(nt + 1) * SPT * S2],
                            ps[:, 0:SPT * S2],
                            bconv[1][:, co:co + 1], 0.0,
                            mybir.AluOpType.add, mybir.AluOpType.max)
                    conv_layer(h1, wconv[1], S1, evac2)

                    def evac3(co, nt, ps, t=t):
                        s0 = t * T + nt * SPT
                        nc.scalar.activation(
                            h3c[co][:, s0 * S3:s0 * S3 + SPT * S3],
                            ps[:, 0:SPT * S3],
                            RELU, bias=bconv[2][:, co:co + 1])
                    conv_layer(h2, wconv[2], S2, evac3)

            # ---------------- FC phase ----------------
            with tc.tile_pool(name="fps", bufs=2, space="PSUM") as fpspool:
                ps_fc1 = [fpspool.tile([P, F1], F32, tag=f"fc1ps{sc}", bufs=1,
                                       name=f"fc1ps{sc}") for sc in range(SC)]
                for sc in range(SC):
                    nc.tensor.matmul(ps_fc1[sc][:], ones[:], bf1[:],
                                     start=True, stop=False)
                for idx in range(len(fc_pairs)):
                    if idx < len(rw_pre):
                        rw = rw_pre[idx]
                    else:
                        rw = wf1_fetch(idx)
                    cc, lg = fc_pairs[idx]
                    for ll in range(GL):
                        l = lg * GL + ll
                        last = (cc == NCHUNK - 1) and (l == LF - 1)
                        for sc in range(SC):
                            nc.tensor.matmul(
                                ps_fc1[sc][:],
                                h3i[cc][:, sc, :, l],
                                rw[:, ll * F1:(ll + 1) * F1],
                                start=False, stop=last)
                # FC2: y = sum_f h4*wf2 + bf2 via one DVE reduce per chunk
                yacc = h4pool.tile([P, SC], F32, tag="yacc", name="yacc")
                yv_d = y_d.ap().rearrange("(p s) one -> p (s one)", s=SC)
                for sc in range(SC):
                    scratch = h4pool.tile([P, F1], F32, tag=f"fc2scr{sc}",
                                          name=f"fc2scr{sc}")
                    if _STTACC:
                        # relu + mult + row-sum in one DVE op from PSUM
                        nc.vector.scalar_tensor_tensor(
                            scratch[:], ps_fc1[sc][:], 0.0, wf2b[:],
                            mybir.AluOpType.max, mybir.AluOpType.mult,
                            accum_out=yacc[:, sc:sc + 1])
                    else:
                        h4 = h4pool.tile([P, F1], F16, tag=f"h4_{sc}",
                                         name=f"h4_{sc}")
                        nc.scalar.activation(h4[:], ps_fc1[sc][:], RELU)
                        nc.vector.tensor_tensor(
                            scratch[:], h4[:], wf2b[:], mybir.AluOpType.mult)
                        nc.vector.tensor_reduce(
                            yacc[:, sc:sc + 1], scratch[:],
                            mybir.AxisListType.X, mybir.AluOpType.add)
                    nc.vector.tensor_scalar_add(
                        yacc[:, sc:sc + 1], yacc[:, sc:sc + 1],
                        bf2t[:, 0:1])
                if _YDMA1:
                    nc.sync.dma_start(yv_d, yacc[:])
                else:
                    for sc in range(SC):
                        nc.sync.dma_start(yv_d[:, sc:sc + 1], yacc[:, sc:sc + 1])

    nc.compile()
    return nc


_NC_CACHE = None


def _get_nc():
    global _NC_CACHE
    if _NC_CACHE is None:
        _NC_CACHE = build_nc()
    return _NC_CACHE


def prep_inputs(x, Wp, bp, W1, b1, W2, b2, W3, b3, Wf1, bf1, Wf2, bf2):
    """Host-side shard + weight re-layout. Returns per-core input maps."""
    f32, f16 = np.float32, np.float16
    wp = np.asarray(Wp, f32)
    wpc = np.ascontiguousarray(
        np.concatenate([wp[:, :, 1].T, wp[:, :, 0].T], axis=0)).astype(f16)
    bpc = np.ascontiguousarray(np.asarray(bp, f32).reshape(NCHUNK, P))

    def conv_t(W):
        # W [co, ci, k] -> [cic, ci, (k, coc, co)] so the DMA is contiguous
        a = np.asarray(W, f32).reshape(NCHUNK, P, NCHUNK, P, 3)
        return np.ascontiguousarray(
            a.transpose(2, 3, 4, 0, 1)).astype(f16).reshape(
                NCHUNK, P, 3 * NCHUNK * P)

    w1t, w2t, w3t = conv_t(W1), conv_t(W2), conv_t(W3)
    b1c = np.ascontiguousarray(np.asarray(b1, f32).reshape(NCHUNK, P))
    b2c = np.ascontiguousarray(np.asarray(b2, f32).reshape(NCHUNK, P))
    b3c = np.ascontiguousarray(np.asarray(b3, f32).reshape(NCHUNK, P))
    # Wf1 [512, 30976] -> [cc, l, c, f] fp16 (contiguous 128KB per (cc, l))
    wf1t = np.ascontiguousarray(
        np.asarray(Wf1, f32).reshape(F1, NCHUNK, P, LF)
        .transpose(1, 3, 2, 0)).astype(f16)
    bf1r = np.ascontiguousarray(np.asarray(bf1, f32).reshape(1, F1)).astype(f16)
    onesr = np.ones((1, P), f16)
    wf2b = np.ascontiguousarray(
        np.broadcast_to(np.asarray(Wf2, f32).reshape(1, F1), (P, F1))).astype(f16)
    bf2c = np.ascontiguousarray(
        np.broadcast_to(np.asarray(bf2, f32).reshape(1, 1), (P, 1)))

    shared = dict(wpc=wpc, bpc=bpc, w1t=w1t, w2t=w2t, w3t=w3t,
                  b1c=b1c, b2c=b2c, b3c=b3c, wf1t=wf1t, bf1r=bf1r,
                  wf2b=wf2b, bf2c=bf2c, onesr=onesr)
    xr = np.asarray(x, f32).reshape(N_CORES, NT, T, CL, IL).astype(f16)
    # [nc, t, j, s, i]: per (t, j) a contiguous 8*128 run -> 64x2KB DMAs
    xtt = np.ascontiguousarray(xr.transpose(0, 1, 4, 2, 3))
    xbt = np.ascontiguousarray(
        np.broadcast_to(xtt[:, :, :, :, 0:1], xtt.shape))           # x0 repl over i
    xth = xtt.reshape(N_CORES, NT, IL, T * CL)
    xbh = xbt.reshape(N_CORES, NT, IL, T * CL)
    return [dict(xth=xth[i], xbh=xbh[i], **shared) for i in range(N_CORES)]


def kernel(x, Wp, bp, W1, b1, W2, b2, W3, b3, Wf1, bf1, Wf2, bf2,
           trace=False, **run_kwargs):
    nc = _get_nc()
    in_maps = prep_inputs(x, Wp, bp, W1, b1, W2, b2, W3, b3, Wf1, bf1, Wf2, bf2)
    res = run_bass_kernel_spmd(nc, in_maps, core_ids=list(range(N_CORES)),
                               trace=trace, **run_kwargs)
    out = np.concatenate([res.results[i]["y"] for i in range(N_CORES)], axis=0)
    kernel.last_results = res
    return out.astype(np.float32)


kernel.last_results = None
